# revision 2
# baseline (speedup 1.0000x reference)
"""Bass kernel v3 for 2-layer LSTM encoder (B=128, T=512, D=128, H=512).

Design: pure data-parallel over batch. Each of the 8 cores runs an IDENTICAL
program on its own 16-sample batch slice — no collectives, no core-id, no
inter-core deps. Both layers run interleaved on every core (L2 lags L1 by
LAG steps), giving two independent dependency chains per core to hide the
per-step ACT/DVE latency behind PE work.

v3 over v2: the input-side (ih) matmul contributions for BOTH layers are
precomputed in 32-step chunks (xg1 = W_ih1 @ x, xg2 = W_ih2 @ out1) so
their weight tiles are loaded once per 512 streamed columns instead of once
per 16 — real TRN2 matmuls are LDWEIGHTS-bound at tiny free dims, which
the CoreSim cost model does not charge for. Each step then injects its xg
column (bias pre-added during the chunk PSUM->SBUF eviction) into the gate
PSUM bank with a single identity matmul that also opens the accumulation
group, followed by the 64 recurrent (hh) matmuls.

On-chip layout (transposed, partitions first):
  x      SBUF [128=D, T*16]            fp16   (t, b) free
  h, c   [128=H-block units, (j, b)]   fp16;  j in 0..3, b in 0..15
  gates  PSUM [128=m-tile units, (m, b)] fp32, m = gate_type*4 + j_out
         gate_type order: i, f, g, o  ->  free blocks [0:64|64:128|128:192|192:256]
  xg     SBUF [128, (m, t, b)] fp16 per 32-step chunk, double buffered

All gates use sigmoid: g-gate rows of W/b are pre-scaled x2 on host and
tanh(g) = 2*sigmoid(2g) - 1 is applied with a fused DVE tensor_scalar.
"""
import sys
sys.path.insert(0, "/opt/trn_rl_repo")
import numpy as np
from concourse import bacc
import concourse.bass as bass
import concourse.mybir as mybir
import concourse.tile as tile

F16 = mybir.dt.float16
F32 = mybir.dt.float32

N_CORES = 8
P = 128
B = 128
D = 128
H = 512
NJ = H // P          # 4 h-blocks
NM = 16              # gate m-tiles (4 gate types x 4 j_out)
BSH = B // N_CORES   # 16 batch per core
GFREE = NM * BSH     # 256 free elems in the gate tile

SIG = mybir.ActivationFunctionType.Sigmoid
TANH = mybir.ActivationFunctionType.Tanh
IDENT_FN = mybir.ActivationFunctionType.Identity
MULT = mybir.AluOpType.mult
SUBTRACT = mybir.AluOpType.subtract

CHUNK = 32           # xg precompute granularity (steps)
CFREE = CHUNK * BSH  # 512 streamed columns per xg production matmul
LAG = 52             # L2 lag behind L1 (chunk + production pipeline)


def build(T, lag=LAG, n_rep=1):
    """n_rep > 1 repeats the whole recurrence (state carried over, x reused)
    inside one program — used only for marginal-time measurement."""
    assert T % CHUNK == 0
    nc = bacc.Bacc()

    x_in = nc.declare_dram_parameter("x_my", [P, T * BSH], F16, isOutput=False)
    whh1_in = nc.declare_dram_parameter("whh1", [P, NJ * NM * P], F16, isOutput=False)
    whh2_in = nc.declare_dram_parameter("whh2", [P, NJ * NM * P], F16, isOutput=False)
    wih1_in = nc.declare_dram_parameter("wih1", [P, NM * P], F16, isOutput=False)
    wih2_in = nc.declare_dram_parameter("wih2", [P, NJ * NM * P], F16, isOutput=False)
    b1_in = nc.declare_dram_parameter("b1t", [P, NM], F32, isOutput=False)
    b2_in = nc.declare_dram_parameter("b2t", [P, NM], F32, isOutput=False)
    id_in = nc.declare_dram_parameter("ident", [P, P], F16, isOutput=False)

    h_out = nc.declare_dram_parameter("h_out", [P, NJ * BSH], F32, isOutput=True)
    c_out = nc.declare_dram_parameter("c_out", [P, NJ * BSH], F32, isOutput=True)

    T_eff = n_rep * T
    n_chunks_eff = T_eff // CHUNK

    with tile.TileContext(nc) as tc:
        with (
            tc.tile_pool(name="wpool", bufs=1) as wpool,
            tc.tile_pool(name="state", bufs=1) as state,
            tc.tile_pool(name="h1p", bufs=4) as h1p,
            tc.tile_pool(name="h2p", bufs=3) as h2p,
            tc.tile_pool(name="actp", bufs=3) as actp,
            tc.tile_pool(name="dvep", bufs=3) as dvep,
            tc.tile_pool(name="xgp", bufs=2) as xgp,
            tc.tile_pool(name="out1p", bufs=2) as out1p,
            tc.tile_pool(name="ps1", bufs=3, space="PSUM") as ps1,
            tc.tile_pool(name="ps2", bufs=3, space="PSUM") as ps2,
            tc.tile_pool(name="psx", bufs=2, space="PSUM") as psx,
        ):
            # ---- load constants ----
            whh1 = wpool.tile([P, NJ * NM * P], F16)
            whh2 = wpool.tile([P, NJ * NM * P], F16)
            wih1 = wpool.tile([P, NM * P], F16)
            wih2 = wpool.tile([P, NJ * NM * P], F16)
            b1t = wpool.tile([P, NM], F32)
            b2t = wpool.tile([P, NM], F32)
            ident = wpool.tile([P, P], F16)
            xt = wpool.tile([P, T * BSH], F16)
            nc.sync.dma_start(out=whh1, in_=whh1_in[:, :])
            nc.sync.dma_start(out=whh2, in_=whh2_in[:, :])
            nc.sync.dma_start(out=wih1, in_=wih1_in[:, :])
            nc.sync.dma_start(out=wih2, in_=wih2_in[:, :])
            nc.sync.dma_start(out=b1t, in_=b1_in[:, :])
            nc.sync.dma_start(out=b2t, in_=b2_in[:, :])
            nc.sync.dma_start(out=ident, in_=id_in[:, :])
            nc.sync.dma_start(out=xt, in_=x_in[:, :])

            c1 = state.tile([P, NJ * BSH], F16)
            c2 = state.tile([P, NJ * BSH], F16)
            hz0 = state.tile([P, NJ * BSH // 2], F16)
            hz1 = state.tile([P, NJ * BSH // 2], F16)
            for t_ in (c1, c2, hz0, hz1):
                nc.vector.memset(t_, 0.0)

            h1_prev = (hz0, hz1)   # halves: j in {0,1} and {2,3}
            h2_prev = (hz0, hz1)

            xg1_chunks = {}
            xg2_chunks = {}
            out1_chunks = {}  # k -> contiguous [P, (j, t, b)] fp16 buffer

            def evict(dst, pt, bias, use_act):
                if use_act:
                    nc.scalar.activation(dst, pt, IDENT_FN, bias=bias)
                else:
                    nc.vector.tensor_scalar_add(dst, pt, bias)

            def produce_xg1_piece(k, m, use_act):
                xgt = xg1_chunks[k]
                t0 = (k % (T // CHUNK)) * CHUNK * BSH
                pt = psx.tile([P, CFREE], F32, tag="xgprod")
                nc.tensor.matmul(
                    pt, wih1[:, m * P:(m + 1) * P],
                    xt[:, t0:t0 + CFREE],
                    start=True, stop=True, skip_group_check=True)
                dst = xgt[:, m, :, :].rearrange("p t b -> p (t b)")
                evict(dst, pt, b1t[:, m:m + 1], use_act)

            def produce_xg2_piece(k, m, use_act):
                """xg2 for chunk k, m-tile m, from the contiguous out1
                chunk buffer: one 512-column matmul per j_in block."""
                ob = out1_chunks[k]  # [P, (j, t, b)]
                pt = psx.tile([P, CFREE], F32, tag="xgprod")
                for j in range(NJ):
                    nc.tensor.matmul(
                        pt, wih2[:, (j * NM + m) * P:(j * NM + m + 1) * P],
                        ob[:, j * CFREE:(j + 1) * CFREE],
                        start=(j == 0), stop=(j == NJ - 1),
                        skip_group_check=True)
                dst = xg2_chunks[k][:, m, :, :].rearrange("p t b -> p (t b)")
                evict(dst, pt, b2t[:, m:m + 1], use_act)

            def gates_pe(psp, tag, xgt, tt, whh, h_prev):
                """PE part of one step: xg inject (opens group, carries
                bias) + 64 hh matmuls, j_in 0,1 before 2,3 per group."""
                Gt = psp.tile([P, GFREE], F32, tag=tag)
                xg_rhs = xgt[:, :, tt, :]  # 3D AP [P, m, b]
                nc.tensor.matmul(Gt[:, :], ident[:, :], xg_rhs,
                                 start=True, stop=False, skip_group_check=True)
                for ms in (range(0, 12), range(12, NM)):
                    for j in range(NJ):
                        hp = h_prev[j // 2]
                        jl = j % 2
                        for m in ms:
                            nc.tensor.matmul(
                                Gt[:, m * BSH:(m + 1) * BSH],
                                whh[:, (j * NM + m) * P:(j * NM + m + 1) * P],
                                hp[:, jl * BSH:(jl + 1) * BSH],
                                start=False, stop=(j == NJ - 1),
                                skip_group_check=True)
                return Gt

            def act_sig(Gt, tag):
                S = actp.tile([P, GFREE], F16, tag=tag)
                nc.scalar.activation(S, Gt, SIG)
                return S

            def dve_c(S, c, tag):
                """g~ = 2*sig(2g)-1; c = f*c + i*g~ (in place), all fp16.
                S layout: i [0:64], f [64:128], g [128:192]."""
                gg = dvep.tile([P, NJ * BSH], F16, tag=tag + "gg")
                nc.vector.tensor_scalar(gg, S[:, 128:192], 2.0, 1.0, MULT, SUBTRACT)
                t2 = dvep.tile([P, NJ * BSH], F16, tag=tag + "t2")
                nc.vector.tensor_mul(t2, S[:, 0:64], gg)
                t1 = dvep.tile([P, NJ * BSH], F16, tag=tag + "t1")
                nc.vector.tensor_mul(t1, S[:, 64:128], c)
                nc.vector.tensor_add(c, t1, t2)

            def act_tanh_c(c, tag):
                tcv = actp.tile([P, NJ * BSH], F16, tag=tag)
                nc.scalar.activation(tcv, c, TANH)
                return tcv

            HB = NJ * BSH // 2  # 32: half of the h free dim (j in {0,1} / {2,3})

            def dve_h(S, tcv, hpool, tag):
                """h written as two independent half tiles so next-step
                j_in{0,1} matmuls can start as soon as half 0 lands.
                o block of S is [192:256]."""
                h0 = hpool.tile([P, HB], F16, tag=tag + "a")
                nc.vector.tensor_mul(h0, S[:, 192:192 + HB], tcv[:, 0:HB])
                h1 = hpool.tile([P, HB], F16, tag=tag + "b")
                nc.vector.tensor_mul(h1, S[:, 192 + HB:256], tcv[:, HB:2 * HB])
                return (h0, h1)

            # ---- prologue: xg1 chunk 0 in full ----
            xg1_chunks[0] = xgp.tile([P, NM, CHUNK, BSH], F16, tag="xg1", name="xg1c")
            for m in range(NM):
                produce_xg1_piece(0, m, use_act=(m % 2 == 0))

            n_iter = T_eff + lag
            for t in range(n_iter):
                do1 = t < T_eff
                do2 = t >= lag

                # paced xg1 production: chunk k during iters [32(k-1)+8..+23]
                if do1:
                    k1 = t // CHUNK + 1
                    ph = t % CHUNK
                    if k1 < n_chunks_eff and 8 <= ph < 8 + NM:
                        m = ph - 8
                        if m == 0:
                            xg1_chunks[k1] = xgp.tile(
                                [P, NM, CHUNK, BSH], F16, tag="xg1", name="xg1c")
                        produce_xg1_piece(k1, m, use_act=(m % 2 == 0))
                # paced xg2 production: chunk k during iters [32k+33..+48]
                if t >= 33:
                    k2, ph2 = divmod(t - 33, CHUNK)
                    if 0 <= k2 < n_chunks_eff and ph2 < NM:
                        m = ph2
                        if m == 0:
                            xg2_chunks[k2] = xgp.tile(
                                [P, NM, CHUNK, BSH], F16, tag="xg2", name="xg2c")
                        produce_xg2_piece(k2, m, use_act=(m % 2 == 1))

                if do1:
                    G1 = gates_pe(ps1, "g1", xg1_chunks[t // CHUNK],
                                  t % CHUNK, whh1, h1_prev)
                if do2:
                    s = t - lag
                    G2 = gates_pe(ps2, "g2", xg2_chunks[s // CHUNK],
                                  s % CHUNK, whh2, h2_prev)
                if do1:
                    S1 = act_sig(G1, "S1")
                if do2:
                    S2 = act_sig(G2, "S2")
                if do1:
                    dve_c(S1, c1, "c1")
                    tc1 = act_tanh_c(c1, "tc1")
                    h1_prev = dve_h(S1, tc1, h1p, "h1")
                    # append h1 into the contiguous out1 chunk buffer for
                    # the xg2 precompute: dst layout (j, t, b)
                    kc, tt = divmod(t, CHUNK)
                    if tt == 0:
                        out1_chunks[kc] = out1p.tile(
                            [P, NJ * CFREE], F16, tag="out1", name="out1c")
                    ob4 = out1_chunks[kc].rearrange(
                        "p (j t b) -> p j t b", j=NJ, t=CHUNK)
                    nc.vector.tensor_copy(
                        ob4[:, 0:2, tt, :],
                        h1_prev[0].rearrange("p (j b) -> p j b", j=2))
                    nc.vector.tensor_copy(
                        ob4[:, 2:4, tt, :],
                        h1_prev[1].rearrange("p (j b) -> p j b", j=2))
                if do2:
                    dve_c(S2, c2, "c2")
                    tc2 = act_tanh_c(c2, "tc2")
                    h2_prev = dve_h(S2, tc2, h2p, "h2")

            h32 = state.tile([P, NJ * BSH], F32)
            nc.vector.tensor_copy(h32[:, 0:HB], h2_prev[0])
            nc.vector.tensor_copy(h32[:, HB:2 * HB], h2_prev[1])
            c32 = state.tile([P, NJ * BSH], F32)
            nc.vector.tensor_copy(c32, c2)
            nc.sync.dma_start(out=h_out[:, :], in_=h32)
            nc.sync.dma_start(out=c_out[:, :], in_=c32)
    return nc


# ---------------- host-side packing ----------------

# m-tile order: gate type (i, f, g, o) major, j_out minor — o LAST so the
# c-path gates i,f,g form the contiguous block [0:192).
# torch row offsets: i:0, f:H, g:2H, o:3H
_GATE_OFF = {0: 0, 1: H, 2: 2 * H, 3: 3 * H}


def _row_perm():
    rows = []
    for m in range(NM):
        g_t, j = divmod(m, NJ)
        base = _GATE_OFF[g_t] + j * P
        rows.extend(range(base, base + P))
    return np.array(rows)


def _pack_w(W, n_cblk, perm):
    """W [2048, C] -> [128, (j_in, m, q)] with lhsT[c, p] = W[row(m,q), j_in*128+c]."""
    Wp = np.asarray(W, np.float32)[perm]                 # [NM*128, C]
    Wr = Wp.reshape(NM, P, n_cblk, P)                    # [m, q, j_in, c]
    out = Wr.transpose(2, 0, 3, 1)                       # [j_in, m, c, q]
    out = out.transpose(2, 0, 1, 3)                      # [c, j_in, m, q]
    return np.ascontiguousarray(out.reshape(P, n_cblk * NM * P)).astype(np.float16)


def pack_inputs(x, W_ih1, W_hh1, b_ih1, b_hh1, W_ih2, W_hh2, b_ih2, b_hh2):
    perm = _row_perm()
    # scale g-gate rows (torch offset 2H..3H) by 2 for the sigmoid-only trick
    def scale_g(W):
        W = np.array(W, np.float32)
        W[2 * H:3 * H] *= 2.0
        return W

    W_ih1 = scale_g(W_ih1); W_hh1 = scale_g(W_hh1)
    W_ih2 = scale_g(W_ih2); W_hh2 = scale_g(W_hh2)
    bias1 = scale_g((np.asarray(b_ih1) + np.asarray(b_hh1)).reshape(-1, 1))[:, 0]
    bias2 = scale_g((np.asarray(b_ih2) + np.asarray(b_hh2)).reshape(-1, 1))[:, 0]

    whh1 = _pack_w(W_hh1, NJ, perm)
    whh2 = _pack_w(W_hh2, NJ, perm)
    wih1 = _pack_w(W_ih1, 1, perm)
    wih2 = _pack_w(W_ih2, NJ, perm)
    # bias transposed for per-partition AP slices: b1t[p, m] = bias[m*128+p]
    b1t = np.ascontiguousarray(bias1[perm].reshape(NM, P).T).astype(np.float32)
    b2t = np.ascontiguousarray(bias2[perm].reshape(NM, P).T).astype(np.float32)
    ident = np.eye(P, dtype=np.float16)

    T = x.shape[1]
    x16 = np.asarray(x).astype(np.float16)
    in_maps = []
    for r in range(N_CORES):
        xs = x16[r * BSH:(r + 1) * BSH]                  # [16, T, 128]
        xm = np.ascontiguousarray(xs.transpose(2, 1, 0).reshape(P, T * BSH))
        in_maps.append({
            "x_my": xm,
            "whh1": whh1, "whh2": whh2, "wih1": wih1, "wih2": wih2,
            "b1t": b1t, "b2t": b2t, "ident": ident,
        })
    return in_maps


def unpack_outputs(results):
    """results: list of 8 dicts with h_out/c_out [128, NJ*BSH] (free = (j, b))."""
    h = np.zeros((B, H), np.float32)
    c = np.zeros((B, H), np.float32)
    for r, res in enumerate(results):
        hr = np.asarray(res["h_out"]).reshape(P, NJ, BSH)   # [p, j, b]
        cr = np.asarray(res["c_out"]).reshape(P, NJ, BSH)
        h[r * BSH:(r + 1) * BSH] = hr.transpose(2, 1, 0).reshape(BSH, H)
        c[r * BSH:(r + 1) * BSH] = cr.transpose(2, 1, 0).reshape(BSH, H)
    return h[None], c[None]


# ---------------- harness entry point ----------------

T_FULL = 512
_CACHE = {}


def _get_nc():
    if "nc" not in _CACHE:
        nc = build(T_FULL)
        nc.finalize()
        _CACHE["nc"] = nc
    return _CACHE["nc"]


def kernel(x, W_ih1, W_hh1, b_ih1, b_hh1, W_ih2, W_hh2, b_ih2, b_hh2):
    import time as _time
    from concourse.bass_utils import run_bass_kernel_spmd
    nc = _get_nc()
    in_maps = pack_inputs(x, W_ih1, W_hh1, b_ih1, b_hh1,
                          W_ih2, W_hh2, b_ih2, b_hh2)
    last_err = None
    for attempt in range(3):
        try:
            res = run_bass_kernel_spmd(nc, in_maps, list(range(N_CORES)))
            return unpack_outputs(res.results)
        except Exception as e:
            last_err = e
            _time.sleep(5 * (attempt + 1))
    raise last_err
